# revision 44
# baseline (speedup 1.0000x reference)
"""Trainium2 Bass kernel for a 2-layer dense GCN (NodeEncoder).

    out = adj @ relu(adj @ (x@W1) + b1) @ W2 + b2
    N=16384, F_IN=512, HID=1024, OUT=256, adj dense [N, N] fp32.

Key algebraic optimization vs the straightforward lowering: layer 1 is
computed as (adj @ x) @ W1 instead of adj @ (x @ W1).  The adj
contraction then runs against F_IN=512 columns instead of HID=1024,
halving the dominant matmul's FLOPs, and since x is replicated on every
core the layer-1 AllGather disappears entirely.

Sharding: adj row-partitioned across 8 NeuronCores (2048 rows/core).
ALL matmuls run fp8 e4m3 with DoubleRow (K=256/instr) and fp32 PSUM:

  phase A:  zT_c   = (adj_c @ x)^T          [512, 2048]
  phase H:  hT_c   = relu(z_c @ W1 + b1)^T  [1024, 2048]
  phase S:  s2_c   = h_c @ W2               [2048, 256]
  AG:       s2     = AllGather(s2_c)        [16384, 256]  (in quarters)
  phase D:  out2T_c = (adj_c @ s2)^T + b2   [256, 2048]

fp8 numerics: quantizing tensors whose rows share a large coherent mean
(z, h) against quantized weights creates rank-1 output error that the
final adj averaging cannot attenuate.  Mitigations (all host-side prep,
validated in numpy to rel err ~3.8e-3 vs the 2e-2 budget):
  - z is mean-centered before fp8: zT evac subtracts zcorr = rank-1
    x-quant correction + colmean(z) (exact, from adj column sums); the
    exact mu@W1 returns via the phase-H ACT bias.
  - h is centered by a Gaussian-CDF estimate h~ of colmean(h) (max err
    ~1% of its rms); the exact h~@W2 returns as an additive correction
    tile at the phase-S evac, so W2-quant error only couples to h - h~.

Perf structure (the kernel is tensor-bound; PE power-throttles to
K=13/16 after ~28us so the only levers are PE cycles and bubbles):
  - first adjA/x transfers are split fine-grained to cut time-to-first-
    real-matmul; phase D's first adjD/s2 tiles prefetch during A(c=1).
  - phase D's last K-step runs bank-major with evac+store inlined per
    bank, overlapping the output tail with the final matmuls.
  - the kernel-exit drain chain is spread across all five engine
    queues instead of serializing on SP.
"""

import numpy as np
import ml_dtypes

import concourse.bass as bass
import concourse.mybir as mybir
import concourse.tile as tile
from concourse.bass_utils import run_bass_kernel_spmd
from concourse.tile_sem_assignment import N_PROCS
from concourse.vector_clock import ScopedClock, VectorClock

# ---------------------------------------------------------------------------
# Workaround: the walrus build in this container caps the number of sync-wait
# commands on a Drain instruction; Tile's kernel-tail drain aggregates one
# wait per logical processor and exceeds it.  Split the tail drain into a
# chain of single-wait drains — distributed round-robin over the five engine
# queues so they retire in parallel, and skipping procs that never ticked.
# ---------------------------------------------------------------------------


def _drain_and_barrier_split(self, tick_clock, wait_clock):
    gc = tick_clock.global_clock
    engines = [
        self.nc.sync,
        self.nc.scalar,
        self.nc.vector,
        self.nc.gpsimd,
        self.nc.tensor,
    ]
    ei = 0
    for p in range(N_PROCS):
        if gc[p] == 0:
            continue
        partial = VectorClock([gc[q] if q == p else 0 for q in range(N_PROCS)])
        d = engines[ei % len(engines)].drain()
        ei += 1
        wait_clock.add_sem_waits(d.ins, ScopedClock({None: partial}))
    self.nc.sync.drain()

    self.nc.all_engine_barrier()
    assert self.sems is not None
    popped = self.nc._tile_sem_poison_stack.pop()
    assert popped is self._sem_poison
    self.nc.clear_and_free_semaphores(list(self.sems.allocated().values()))
    self.nc.all_engine_barrier()


tile.TileContext._drain_and_barrier = _drain_and_barrier_split

# The same walrus cap applies to every instruction kind: at most ONE sync
# wait command per instruction (probed empirically — a 2-wait TensorCopy is
# rejected).  Post-pass: hoist excess sem-waits onto no-ops inserted just
# before the instruction on the same engine queue — per-engine program order
# makes this semantically identical.
_MAX_WAITS = 1


def _split_excess_waits(nc):
    ctr = 0
    for f in nc.m.functions:
        for bb in f.blocks:
            out = []
            changed = False
            for inst in bb.instructions:
                si = inst.sync_info
                waits = list(si.on_wait) if si is not None and si.on_wait else []
                if len(waits) > _MAX_WAITS:
                    changed = True
                    keep, excess = waits[: _MAX_WAITS], waits[_MAX_WAITS :]
                    for i in range(0, len(excess), _MAX_WAITS):
                        ctr += 1
                        nop = mybir.InstNoOp(name=f"I-waitnop-{ctr}")
                        nop.engine = inst.engine
                        nop.sync_info = mybir.SyncInfo(
                            on_wait=excess[i : i + _MAX_WAITS], on_update=[]
                        )
                        out.append(nop)
                    si.on_wait = keep
                out.append(inst)
            if changed:
                bb.instructions = out
    return ctr


def _elide_redundant_ldweights(nc):
    """Delete an InstLdweights that reloads the exact weights AP loaded by
    the previous (surviving) InstLdweights when only plain matmuls / no-ops
    sit between them in the scheduled stream.  The PE array keeps the
    stationary operand across matmuls, so the reload is pure overhead
    (walrus emits one LDWEIGHTS per MATMUL and its ldw-opt pass is
    incompatible with pre-split LDW+MM).  Only sync-free LDWs are removed,
    so semaphore bookkeeping is unchanged."""
    n_elided = 0
    for f in nc.m.functions:
        for bb in f.blocks:
            out = []
            last_w = None  # weights-AP repr of last surviving LDW, if run intact
            changed = False
            for inst in bb.instructions:
                nm = type(inst).__name__
                if nm == "InstLdweights":
                    si = inst.sync_info
                    clean = not (si and (si.on_wait or si.on_update))
                    w = repr(inst.ins[0])
                    if clean and last_w == w:
                        n_elided += 1
                        changed = True
                        continue  # drop the reload
                    last_w = w if clean else None
                elif nm == "InstMatmult":
                    if getattr(inst, "is_transpose", False):
                        last_w = None
                elif nm == "InstNoOp":
                    pass
                else:
                    last_w = None
                out.append(inst)
            if changed:
                bb.instructions = out
    return n_elided


NCORES = 8
N = 16384
SH = N // NCORES  # 2048 adj rows per core
F = 512
HID = 1024
OUT = 256

BF16 = mybir.dt.bfloat16
F32 = mybir.dt.float32
FP8 = mybir.dt.float8e4
ADJ_SCALE = float(N)  # adj pre-scaled into fp8 range; 1/N folded downstream
S2_SCALE = 1024.0  # s2 pre-scaled into fp8 range; undone at phase D evac
S_Z = 0.5  # zT fp8 scale (centered z, max ~100 < 240)
S_W1 = 64.0  # W1 fp8 scale
S_H = 8192.0  # centered-h fp8 scale (max ~126 < 240)
S_W2 = 64.0  # W2 fp8 scale

DR = mybir.MatmulPerfMode.DoubleRow

_built = None


def build():
    """Build the per-core Bass program (identical on all cores)."""
    nc = bass.Bass()

    # All big inputs are host-prepped into partition-major tiled layouts so
    # every DMA reads long contiguous per-partition runs (8-64 KiB):
    #   adjD[p, (k4 kk m)] = adjT[k4*512+kk*128+p, m]  (phases A and D)
    #   xP  [p, (kb f)]    = x[kb*128+p, f]            (replicated)
    adjA = nc.declare_dram_parameter("adjA", [128, 2 * 32 * 4 * 1024], FP8, isOutput=False)
    adjD = nc.declare_dram_parameter("adjD", [128, 32 * 4 * SH], FP8, isOutput=False)
    xP = nc.declare_dram_parameter("xP", [128, (N // 128) * F], FP8, isOutput=False)
    w1 = nc.declare_dram_parameter("w1", [F, HID], FP8, isOutput=False)
    w2 = nc.declare_dram_parameter("w2", [HID, OUT], FP8, isOutput=False)
    b1T = nc.declare_dram_parameter("b1T", [128, HID // 128], F32, isOutput=False)
    b2T = nc.declare_dram_parameter("b2T", [128, OUT // 128], F32, isOutput=False)
    # per-element subtraction at the zT evac: rank-1 x-quant correction
    # m_f*R_m PLUS the (broadcast) column mean of z, so the fp8 zT holds only
    # the zero-mean residual.  The exact mean returns via the phase-H bias.
    # bf16 (rounding noise ~0.04% of pre-act sigma): at 4 MB the fp32 version
    # monopolized HBM in the front-loaded first chunk and starved the adjA
    # stream.
    zcorr = nc.declare_dram_parameter("zcorr", [128, (F // 128) * SH], BF16, isOutput=False)
    # per-partition S_H*h~ (colmean-of-h estimate), subtracted after relu
    hcorrT = nc.declare_dram_parameter("hcorrT", [128, HID // 128], F32, isOutput=False)
    # S2_SCALE * (h~ @ W2), replicated across partitions, added at phase-S evac
    scorrT = nc.declare_dram_parameter("scorrT", [128, OUT], F32, isOutput=False)
    # [n2t, mb, p, m]: per-psum-bank contiguous so the final stores hit DMA
    # line rate (the [OUT, SH] layout made every partition row a strided
    # 2 KB piece and the stores trickled for ~5us past the last matmul)
    out2T = nc.declare_dram_parameter("out2T", [2, 4, 128, 512], F32, isOutput=True)

    rg = [list(range(NCORES))]

    def allgather(inp, outp):
        return nc.gpsimd.collective_compute(
            "AllGather",
            mybir.AluOpType.bypass,
            replica_groups=rg,
            ins=[inp.opt()],
            outs=[outp.opt()],
        )

    with tile.TileContext(nc) as tc:
        with (
            tc.tile_pool(name="const", bufs=1) as constp,
            tc.tile_pool(name="psum", bufs=8, space="PSUM") as psum,
            tc.tile_pool(name="dram", bufs=1, space="DRAM") as dram,
            tc.tile_pool(name="s2p", bufs=6) as s2p,
            tc.tile_pool(name="adjD", bufs=4) as adjDp,
        ):
            # ---- constants (ACT HWDGE ring; adj streams ride the SP ring).
            # Declared here, but the DMAs are issued AFTER the first x tiles
            # below: phase A's first matmul gates on x tile 0, while the
            # weights aren't read until phase H much later.
            w1t = constp.tile([128, F // 128, HID], FP8)
            w2t = constp.tile([128, HID // 128, OUT], FP8)
            b1t = constp.tile([128, HID // 128], F32)
            b2t = constp.tile([128, OUT // 128], F32)
            hct = constp.tile([128, HID // 128], F32)
            sct = constp.tile([128, OUT], F32)
            zct = constp.tile([128, 4, F // 128, 512], BF16)  # [p, chunk, fb, m]
            wz = constp.tile([128, 2, 512], FP8)  # zeroed warm-up operand

            # AG buffers partition-major: rank contribution [128, skk*256+n]
            # with s2 row skk*128+p; gathered output stacks ranks on dim 0.
            # Collectives cost 30-60us nearly size-independent AND serialize
            # on the single CC stream, so one quarter-gather fires per
            # m-chunk (~41us apart): even worst-case collectives drain
            # before phase D's quarter-major consumption needs them.
            ag_in = [dram.tile([128, 4 * OUT], FP8, name=f"agi{q}") for q in range(4)]
            ag_out = [
                dram.tile([128 * 8, 4 * OUT], FP8, addr_space="Shared", name=f"ago{q}")
                for q in range(4)
            ]

            # xP holds SW-interleaved stationary blocks for phase A:
            #   [p, q, fb, (127-c)*2 + ko] = x_hi[q*256 + ko*128 + p, fb*128 + c]
            xsrc = xP[:].rearrange("p (q fb c) -> p q fb c", q=N // 256, fb=F // 128)

            # ag_out readers: [p, g, skk, n]; s2 global row for quarter qq
            # is g*2048 + qq*512 + skk*128 + p
            s2srcs = [
                ag_out[qq][:].rearrange("(g p) (skk n) -> p g skk n", p=128, n=OUT)
                for qq in range(4)
            ]
            dsrc = adjD[:].rearrange("p (k4 kk m) -> p k4 kk m", k4=32, kk=4)
            # phase D k-step order: quarter-major, matching AG firing order
            k4_order = [g * 4 + qq for qq in range(4) for g in range(8)]
            st_pre = {}
            ad_pre = {}

            with (
                tc.tile_pool(name="xp", bufs=1) as xp,
                tc.tile_pool(name="zt", bufs=16) as ztp,
                tc.tile_pool(name="ht", bufs=16) as htp,
                tc.tile_pool(name="htmp", bufs=4) as htmpp,
                tc.tile_pool(name="adjA", bufs=6) as adjp,
                tc.tile_pool(name="small", bufs=4) as smallp,
            ):
                # ---- PE warm-up: the engine preamble ends ~10.5us and the
                # first adjA/x bytes land ~14us; 7 throwaway fp8-DR matmuls
                # on a zeroed tile fill that window and lift the HAM clock
                # gate (needs ~3.4us of sustained PE busy) so the leading
                # real matmuls run at 2.4 GHz instead of 1.2.  Results land
                # in psum banks phase A's start=True matmuls overwrite.
                # memset rides the otherwise-idle GpSimd queue.
                with tc.high_priority():
                    nc.gpsimd.memset(wz[:], 0)
                    wps = [
                        psum.tile([128, 512], F32, tag="ps", name=f"psW{i}")
                        for i in range(2)
                    ]
                    for i in range(7):
                        nc.tensor.matmul(
                            wps[i % 2][:],
                            wz[:, :, 0:128],
                            wz[:],
                            start=True,
                            stop=True,
                            perf_mode=DR,
                        )

                xts = []  # 16 tiles of 8 k-blocks each
                zt = {}
                ht = {}
                aAv = adjA[:].rearrange(
                    "p (c k4 kk m) -> p c k4 kk m", c=4, k4=32, kk=4
                )
                act_scale = S_H / (S_Z * S_W1 * ADJ_SCALE)
                s_evac = S2_SCALE / (S_H * S_W2)
                for ch in range(4):
                    asrc = aAv[:, ch]
                    # ---- phase A: zT chunk = (adj_c @ x)^T cols ch*512.. --
                    ps = [
                        psum.tile([128, 512], F32, tag="ps", name=f"psA{ch}{i}")
                        for i in range(4)
                    ]
                    for k4 in range(32):
                        if ch == 0 and k4 % 2 == 0:
                            i = k4 // 2
                            t = xp.tile([128, 4, 4, 256], FP8, name=f"xt{i}")
                            if i == 0:
                                # fine-grained first transfer: the leading
                                # matmuls need only q=0
                                for qi in range(4):
                                    nc.scalar.dma_start(t[:, qi], xsrc[:, qi])
                            else:
                                nc.scalar.dma_start(
                                    t[:], xsrc[:, i * 4 : (i + 1) * 4]
                                )
                            xts.append(t)
                            if i % 4 == 1:
                                # one 512 KB zcorr chunk-piece rides between
                                # x tiles: as one trailing 2-4 MB transfer it
                                # finished ~9us after the first chunk's evac
                                # needed it, stalling the whole pipeline
                                zch = (i - 1) // 4
                                nc.scalar.dma_start(
                                    zct[:, zch],
                                    zcorr[:].rearrange(
                                        "p (c fb mm) -> p c fb mm", c=4, fb=F // 128
                                    )[:, zch],
                                )
                            if i == 15:
                                # x fully queued; now the weight constants
                                nc.scalar.dma_start(
                                    w1t[:],
                                    w1[:].rearrange("(fb p) j -> p fb j", p=128),
                                )
                                nc.scalar.dma_start(
                                    w2t[:],
                                    w2[:].rearrange("(jb p) n -> p jb n", p=128),
                                )
                                nc.scalar.dma_start(b1t[:], b1T[:])
                                nc.scalar.dma_start(b2t[:], b2T[:])
                                nc.scalar.dma_start(hct[:], hcorrT[:])
                                nc.scalar.dma_start(sct[:], scorrT[:])
                        if ch == 3 and k4 == 16:
                            # prefetch phase D's first four s2 tiles (g 0..3
                            # of quarter 0).  That AllGather completed long
                            # ago, so the SP queue won't block, and phase
                            # D's first matmuls start without a JIT-load
                            # bubble.
                            for pk in range(4):
                                stp = s2p.tile(
                                    [128, 4, OUT], FP8, tag="s2t", bufs=6,
                                    name=f"s2tp{pk}",
                                )
                                nc.sync.dma_start(stp[:], s2srcs[0][:, pk])
                                st_pre[pk] = stp
                        if ch == 3 and k4 in (20, 26):
                            # prefetch phase D's first adjD tiles: the ACT
                            # engine reaches these dma_starts mid-A (its
                            # queue is idle then), so the 1 MB loads land
                            # well before phase D starts instead of being
                            # issued behind the last H evacs.
                            pk = 0 if k4 == 20 else 1
                            adp = adjDp.tile(
                                [128, 4, SH], FP8, tag="adjD", bufs=4,
                                name=f"aDp{pk}",
                            )
                            nc.scalar.dma_start(adp[:], dsrc[:, k4_order[pk]])
                            ad_pre[pk] = adp
                        at = adjp.tile(
                            [128, 4, 512], FP8, tag="adjA", bufs=6, name=f"aA{ch}{k4}"
                        )
                        if ch == 0 and k4 < 2:
                            # fine-grained: j2=0 matmuls need only kk 0:2
                            nc.sync.dma_start(at[:, 0:2], asrc[:, k4, 0:2])
                            nc.sync.dma_start(at[:, 2:4], asrc[:, k4, 2:4])
                        else:
                            nc.sync.dma_start(at[:], asrc[:, k4])
                        # fp8 DoubleRow: contraction 256 rows per matmul
                        # (ki = partition, ko = kk-pair), 2x FLOP rate.
                        for j2 in range(2):
                            q = k4 * 2 + j2
                            xt = xts[q // 4]
                            for fb in range(4):
                                nc.tensor.matmul(
                                    ps[fb][:],
                                    xt[:, q % 4, fb, :],
                                    at[:, 2 * j2 : 2 * j2 + 2, :],
                                    start=(q == 0),
                                    stop=(q == 63),
                                    perf_mode=mybir.MatmulPerfMode.DoubleRowSwInterleave,
                                )
                    for fb in range(4):
                        if fb % 2 == 0:
                            zz = ztp.tile(
                                [128, 2, 512], FP8, tag="zt", bufs=16,
                                name=f"zt{ch}{fb}",
                            )
                            zt[(fb // 2, ch)] = zz
                        # fp8 zT holds S_Z*(psum - zcorr): zero-mean
                        # residual of z, coherent means handled exactly
                        nc.vector.scalar_tensor_tensor(
                            zt[(fb // 2, ch)][:, fb % 2, :],
                            ps[fb][:],
                            S_Z,
                            zct[:, ch, fb, :],
                            mybir.AluOpType.mult,
                            mybir.AluOpType.subtract,
                        )

                    # ---- phase H: hT chunk = relu(z @ W1 + b1)^T, fp8 DR --
                    # psum_H = (S_Z*deltaN) @ (S_W1*W1) = S_Z*S_W1*N*(delta@W1)
                    for jbh in range(2):
                        psh = [
                            psum.tile([128, 512], F32, tag="ps", name=f"psH{ch}{jbh}{i}")
                            for i in range(4)
                        ]
                        for jb in range(4):
                            jg = jbh * 4 + jb
                            for fbp in range(2):
                                nc.tensor.matmul(
                                    psh[jb][:],
                                    w1t[:, 2 * fbp : 2 * fbp + 2, jg * 128 : (jg + 1) * 128],
                                    zt[(fbp, ch)][:],
                                    start=(fbp == 0),
                                    stop=(fbp == 1),
                                    perf_mode=DR,
                                )
                            # tmp = S_H*relu(...): ACT does scale+bias+relu;
                            # DVE then subtracts S_H*h~ and casts to fp8
                            tmp = htmpp.tile(
                                [128, 512], BF16, tag="htmp", bufs=4,
                                name=f"htm{ch}{jbh}{jb}",
                            )
                            nc.scalar.activation(
                                tmp[:],
                                psh[jb][:],
                                mybir.ActivationFunctionType.Relu,
                                bias=b1t[:, jg : jg + 1],
                                scale=act_scale,
                            )
                            if jg % 2 == 0:
                                hh = htp.tile(
                                    [128, 2, 512], FP8, tag="ht", bufs=16,
                                    name=f"ht{ch}{jbh}{jb}",
                                )
                                ht[(jg // 2, ch)] = hh
                            nc.vector.tensor_scalar(
                                ht[(jg // 2, ch)][:, jg % 2, :],
                                tmp[:],
                                1.0,
                                hct[:, jg : jg + 1],
                                mybir.AluOpType.mult,
                                mybir.AluOpType.subtract,
                            )

                    # ---- phase S: s2 chunk = h @ W2, fp8 DR; AG/quarter ---
                    # psum_S = S_H*S_W2*((h-h~)@W2); evac adds S2_SCALE*h~@W2
                    for m4 in range(4):
                        pss = psum.tile([128, 256], F32, tag="ps", name=f"psS{ch}{m4}")
                        for jp in range(4):
                            nc.tensor.matmul(
                                pss[:],
                                ht[(jp, ch)][:, :, m4 * 128 : (m4 + 1) * 128],
                                w2t[:, 2 * jp : 2 * jp + 2, :],
                                start=(jp == 0),
                                stop=(jp == 3),
                                perf_mode=DR,
                            )
                        so = smallp.tile([128, 256], FP8, tag="so", bufs=4)
                        nc.vector.scalar_tensor_tensor(
                            so[:],
                            pss[:],
                            s_evac,
                            sct[:],
                            mybir.AluOpType.mult,
                            mybir.AluOpType.add,
                        )
                        # ACT HWDGE ring: the gpsimd queue must stay empty
                        # of data movement — a collective trigger BLOCKS it
                        # until the CC stream is free, and anything queued
                        # behind cascades into a PE stall.
                        nc.scalar.dma_start(
                            ag_in[ch][:, m4 * OUT : (m4 + 1) * OUT], so[:]
                        )
                    allgather(ag_in[ch], ag_out[ch])

            # ---- phase D: out2T = (adj_c @ s2)^T + b2 ----
            # All 8 psum banks accumulate concurrently; k-blocks consumed in
            # gather-arrival order (quarter-major), s2 tiles loaded JIT after
            # each adjT chunk so the SP queue stays load-ordered.
            with (
                tc.tile_pool(name="outp", bufs=8) as outp,
            ):
                dps = [
                    psum.tile([128, 512], F32, tag="ps", name=f"psD{i}")
                    for i in range(8)
                ]
                inv = 1.0 / (ADJ_SCALE * S2_SCALE)

                def d_evac(n2t, mb):
                    # scalar and vector engines each take half (they can
                    # access PSUM concurrently on different banks), stores
                    # split across both HWDGE rings
                    ot = outp.tile([128, 512], F32, tag="ot")
                    if mb % 2 == 0:
                        nc.scalar.activation(
                            ot[:],
                            dps[n2t * 4 + mb][:],
                            mybir.ActivationFunctionType.Identity,
                            bias=b2t[:, n2t : n2t + 1],
                            scale=inv,
                        )
                    else:
                        nc.vector.tensor_scalar(
                            ot[:],
                            dps[n2t * 4 + mb][:],
                            inv,
                            b2t[:, n2t : n2t + 1],
                            mybir.AluOpType.mult,
                            mybir.AluOpType.add,
                        )
                    dmaq = nc.scalar if mb % 2 == 0 else nc.sync
                    dmaq.dma_start(out2T[n2t, mb], ot[:])

                for ki, k4 in enumerate(k4_order):
                    g, qq = k4 // 4, k4 % 4
                    if ki in ad_pre:
                        at = ad_pre[ki]
                    else:
                        at = adjDp.tile(
                            [128, 4, SH], FP8, tag="adjD", bufs=4, name=f"aD{k4}"
                        )
                        nc.scalar.dma_start(at[:], dsrc[:, k4])
                    if ki in st_pre:
                        st = st_pre[ki]
                    else:
                        st = s2p.tile(
                            [128, 4, OUT], FP8, tag="s2t", bufs=6, name=f"s2t{k4}"
                        )
                        nc.sync.dma_start(st[:], s2srcs[qq][:, g])
                    if ki < 31:
                        for j2 in range(2):
                            for n2t in range(2):
                                lhs = st[:, 2 * j2 : 2 * j2 + 2, n2t * 128 : (n2t + 1) * 128]
                                for mb in range(4):
                                    nc.tensor.matmul(
                                        dps[n2t * 4 + mb][:],
                                        lhs,
                                        at[:, 2 * j2 : 2 * j2 + 2, mb * 512 : (mb + 1) * 512],
                                        start=(ki == 0 and j2 == 0),
                                        stop=False,
                                        perf_mode=DR,
                                    )
                    else:
                        # final K-step bank-major: each bank finishes its
                        # accumulation then evacs+stores immediately, hiding
                        # the output tail under the remaining matmuls
                        for n2t in range(2):
                            for mb in range(4):
                                for j2 in range(2):
                                    lhs = st[:, 2 * j2 : 2 * j2 + 2, n2t * 128 : (n2t + 1) * 128]
                                    nc.tensor.matmul(
                                        dps[n2t * 4 + mb][:],
                                        lhs,
                                        at[:, 2 * j2 : 2 * j2 + 2, mb * 512 : (mb + 1) * 512],
                                        start=False,
                                        stop=(j2 == 1),
                                        perf_mode=DR,
                                    )
                                d_evac(n2t, mb)

    _elide_redundant_ldweights(nc)
    _split_excess_waits(nc)
    return nc


def _erf(z):
    """Abramowitz-Stegun 7.1.26 erf approximation (|err| < 1.5e-7)."""
    s = np.sign(z)
    z = np.abs(z)
    t = 1.0 / (1.0 + 0.3275911 * z)
    poly = t * (
        0.254829592
        + t * (-0.284496736 + t * (1.421413741 + t * (-1.453152027 + t * 1.061405429)))
    )
    return s * (1.0 - poly * np.exp(-z * z))


def _prep_inputs(x, adj, W1, b1, W2, b2):
    e4 = ml_dtypes.float8_e4m3

    def q8(a):
        return np.clip(a, -240.0, 240.0).astype(e4)

    # Phase A runs in fp8 (DoubleRow): adj pre-scaled by N into e4m3 range,
    # 1/N folded downstream.  x quantized to e4m3; the coherent part of its
    # quantization error (rank-1: adjq_rowsum x colmean(x_hi - x)) is
    # cancelled in the zT evac together with the z column mean.
    x_hi = q8(x)
    x_hi32 = x_hi.astype(np.float32)
    # SW-interleaved stationary layout for DoubleRowSwInterleave:
    #   xP[p, q, fb, (127-c)*2+ko] = x_hi[q*256 + ko*128 + p, fb*128 + c]
    a5x = x_hi.reshape(N // 256, 2, 128, F // 128, 128)  # [q, ko, ki, fb, c]
    xb = np.ascontiguousarray(
        a5x.transpose(2, 0, 3, 4, 1)[:, :, :, ::-1, :]
    ).reshape(128, -1)
    m = (x_hi32 - x).mean(axis=0)  # [F]
    b2T = np.ascontiguousarray(b2.reshape(OUT // 128, 128).T).astype(np.float32)

    adjTqs = []
    colsum = np.zeros(N, dtype=np.float64)  # sum over adj ROWS of adjq [k]
    rrsum = 0.0
    for c in range(NCORES):
        rows = slice(c * SH, (c + 1) * SH)
        # adjT[k, m] = adj[c*SH + m, k], shape [N, SH], k-major
        adjTq = q8(np.ascontiguousarray(adj[rows, :].T) * ADJ_SCALE)
        adjTqs.append(adjTq)
        aq32 = adjTq.astype(np.float32)
        colsum += aq32.sum(axis=1, dtype=np.float64)
        rrsum += float(aq32.sum(dtype=np.float64))
    # muN[f] = colmean over all rows of (adjq@xq - rr*m)  (N-scaled z)
    muN = (colsum / N).astype(np.float32) @ x_hi32 - m * np.float32(rrsum / N)

    # phase-H effective bias (true scale) and its Gaussian h-mean estimate
    bias_eff = (muN / ADJ_SCALE) @ W1 + b1  # [HID]
    var_delta_f = (1.0 / 12.0) * (x * x).sum(axis=0) / (ADJ_SCALE * ADJ_SCALE)
    sig_j = np.sqrt(np.maximum(var_delta_f @ (W1 * W1), 1e-30))  # [HID]
    t = bias_eff / sig_j
    phi_cdf = 0.5 * (1.0 + _erf(t / np.sqrt(2.0)))
    phi_pdf = np.exp(-0.5 * t * t) / np.sqrt(2.0 * np.pi)
    h_tilde = bias_eff * phi_cdf + sig_j * phi_pdf  # ~ colmean(relu(pre))

    b1T_eff = np.ascontiguousarray(
        (S_H * bias_eff).reshape(HID // 128, 128).T
    ).astype(np.float32)
    hcorrT = np.ascontiguousarray(
        (S_H * h_tilde).reshape(HID // 128, 128).T
    ).astype(np.float32)
    scorr = (S2_SCALE * (h_tilde @ W2)).astype(np.float32)  # [OUT]
    scorrT = np.broadcast_to(scorr, (128, OUT)).copy()

    w1q = q8(S_W1 * W1)
    w2q = q8(S_W2 * W2)

    in_maps = []
    for c in range(NCORES):
        adjTq = adjTqs[c]
        a5 = adjTq.reshape(32, 4, 128, 4, 512)  # [k4, kk, p, chunk, m]
        adjA_ = np.ascontiguousarray(a5.transpose(2, 3, 0, 1, 4)).reshape(128, -1)
        d4 = adjTq.reshape(32, 4, 128, SH)  # [k4, kk, p, m]
        adjD_ = np.ascontiguousarray(d4.transpose(2, 0, 1, 3)).reshape(128, -1)
        rr = adjTq.astype(np.float32).sum(axis=0)  # [SH] adjq rowsums
        # zcorr = S_Z*(m_f*rr_m + muN_f): x-quant rank-1 + z column mean
        # layout [p, chunk, fb, m512] so each chunk's piece is contiguous
        zc = (S_Z * (m[:, None] * rr[None, :] + muN[:, None])).reshape(
            F // 128, 128, 4, 512
        )
        zcorr_ = (
            np.ascontiguousarray(zc.transpose(1, 2, 0, 3)).reshape(128, -1)
        ).astype(ml_dtypes.bfloat16)
        in_maps.append(
            {
                "adjA": adjA_,
                "adjD": adjD_,
                "xP": xb,
                "w1": w1q,
                "w2": w2q,
                "b1T": b1T_eff,
                "b2T": b2T,
                "zcorr": zcorr_,
                "hcorrT": hcorrT,
                "scorrT": scorrT,
            }
        )
    return in_maps


def _run(inputs, trace=False):
    global _built
    if _built is None:
        _built = build()
    in_maps = _prep_inputs(**inputs)
    r = run_bass_kernel_spmd(_built, in_maps, list(range(NCORES)), trace=trace)
    out = np.empty([N, OUT], np.float32)
    for c in range(NCORES):
        # out2T[n2t, mb, p, m] = out[c*SH + mb*512 + m, n2t*128 + p]
        o4 = r.results[c]["out2T"]
        out[c * SH : (c + 1) * SH, :] = o4.transpose(1, 3, 0, 2).reshape(SH, OUT)
    return out, r


def kernel(x, adj, W1, b1, W2, b2):
    out, _ = _run(dict(x=x, adj=adj, W1=W1, b1=b1, W2=W2, b2=b2))
    return out


# revision 45
# speedup vs baseline: 1.0039x; 1.0039x over previous
"""Trainium2 Bass kernel for a 2-layer dense GCN (NodeEncoder).

    out = adj @ relu(adj @ (x@W1) + b1) @ W2 + b2
    N=16384, F_IN=512, HID=1024, OUT=256, adj dense [N, N] fp32.

Key algebraic optimization vs the straightforward lowering: layer 1 is
computed as (adj @ x) @ W1 instead of adj @ (x @ W1).  The adj
contraction then runs against F_IN=512 columns instead of HID=1024,
halving the dominant matmul's FLOPs, and since x is replicated on every
core the layer-1 AllGather disappears entirely.

Sharding: adj row-partitioned across 8 NeuronCores (2048 rows/core).
ALL matmuls run fp8 e4m3 with DoubleRow (K=256/instr) and fp32 PSUM:

  phase A:  zT_c   = (adj_c @ x)^T          [512, 2048]
  phase H:  hT_c   = relu(z_c @ W1 + b1)^T  [1024, 2048]
  phase S:  s2_c   = h_c @ W2               [2048, 256]
  AG:       s2     = AllGather(s2_c)        [16384, 256]  (in quarters)
  phase D:  out2T_c = (adj_c @ s2)^T + b2   [256, 2048]

fp8 numerics: quantizing tensors whose rows share a large coherent mean
(z, h) against quantized weights creates rank-1 output error that the
final adj averaging cannot attenuate.  Mitigations (all host-side prep,
validated in numpy to rel err ~3.8e-3 vs the 2e-2 budget):
  - z is mean-centered before fp8: zT evac subtracts zcorr = rank-1
    x-quant correction + colmean(z) (exact, from adj column sums); the
    exact mu@W1 returns via the phase-H ACT bias.
  - h is centered by a Gaussian-CDF estimate h~ of colmean(h) (max err
    ~1% of its rms); the exact h~@W2 returns as an additive correction
    tile at the phase-S evac, so W2-quant error only couples to h - h~.

Perf structure (the kernel is tensor-bound; PE power-throttles to
K=13/16 after ~28us so the only levers are PE cycles and bubbles):
  - first adjA/x transfers are split fine-grained to cut time-to-first-
    real-matmul; phase D's first adjD/s2 tiles prefetch during A(c=1).
  - phase D's last K-step runs bank-major with evac+store inlined per
    bank, overlapping the output tail with the final matmuls.
  - the kernel-exit drain chain is spread across all five engine
    queues instead of serializing on SP.
"""

import numpy as np
import ml_dtypes

import concourse.bass as bass
import concourse.mybir as mybir
import concourse.tile as tile
from concourse.bass_utils import run_bass_kernel_spmd
from concourse.tile_sem_assignment import N_PROCS
from concourse.vector_clock import ScopedClock, VectorClock

# ---------------------------------------------------------------------------
# Workaround: the walrus build in this container caps the number of sync-wait
# commands on a Drain instruction; Tile's kernel-tail drain aggregates one
# wait per logical processor and exceeds it.  Split the tail drain into a
# chain of single-wait drains — distributed round-robin over the five engine
# queues so they retire in parallel, and skipping procs that never ticked.
# ---------------------------------------------------------------------------


def _drain_and_barrier_split(self, tick_clock, wait_clock):
    gc = tick_clock.global_clock
    engines = [
        self.nc.sync,
        self.nc.scalar,
        self.nc.vector,
        self.nc.gpsimd,
        self.nc.tensor,
    ]
    ei = 0
    for p in range(N_PROCS):
        if gc[p] == 0:
            continue
        partial = VectorClock([gc[q] if q == p else 0 for q in range(N_PROCS)])
        d = engines[ei % len(engines)].drain()
        ei += 1
        wait_clock.add_sem_waits(d.ins, ScopedClock({None: partial}))
    self.nc.sync.drain()

    self.nc.all_engine_barrier()
    assert self.sems is not None
    popped = self.nc._tile_sem_poison_stack.pop()
    assert popped is self._sem_poison
    self.nc.clear_and_free_semaphores(list(self.sems.allocated().values()))
    self.nc.all_engine_barrier()


tile.TileContext._drain_and_barrier = _drain_and_barrier_split

# The same walrus cap applies to every instruction kind: at most ONE sync
# wait command per instruction (probed empirically — a 2-wait TensorCopy is
# rejected).  Post-pass: hoist excess sem-waits onto no-ops inserted just
# before the instruction on the same engine queue — per-engine program order
# makes this semantically identical.
_MAX_WAITS = 1


def _split_excess_waits(nc):
    ctr = 0
    for f in nc.m.functions:
        for bb in f.blocks:
            out = []
            changed = False
            for inst in bb.instructions:
                si = inst.sync_info
                waits = list(si.on_wait) if si is not None and si.on_wait else []
                if len(waits) > _MAX_WAITS:
                    changed = True
                    keep, excess = waits[: _MAX_WAITS], waits[_MAX_WAITS :]
                    for i in range(0, len(excess), _MAX_WAITS):
                        ctr += 1
                        nop = mybir.InstNoOp(name=f"I-waitnop-{ctr}")
                        nop.engine = inst.engine
                        nop.sync_info = mybir.SyncInfo(
                            on_wait=excess[i : i + _MAX_WAITS], on_update=[]
                        )
                        out.append(nop)
                    si.on_wait = keep
                out.append(inst)
            if changed:
                bb.instructions = out
    return ctr


def _elide_redundant_ldweights(nc):
    """Delete an InstLdweights that reloads the exact weights AP loaded by
    the previous (surviving) InstLdweights when only plain matmuls / no-ops
    sit between them in the scheduled stream.  The PE array keeps the
    stationary operand across matmuls, so the reload is pure overhead
    (walrus emits one LDWEIGHTS per MATMUL and its ldw-opt pass is
    incompatible with pre-split LDW+MM).  Only sync-free LDWs are removed,
    so semaphore bookkeeping is unchanged."""
    n_elided = 0
    for f in nc.m.functions:
        for bb in f.blocks:
            out = []
            last_w = None  # weights-AP repr of last surviving LDW, if run intact
            changed = False
            for inst in bb.instructions:
                nm = type(inst).__name__
                if nm == "InstLdweights":
                    si = inst.sync_info
                    clean = not (si and (si.on_wait or si.on_update))
                    w = repr(inst.ins[0])
                    if clean and last_w == w:
                        n_elided += 1
                        changed = True
                        continue  # drop the reload
                    last_w = w if clean else None
                elif nm == "InstMatmult":
                    if getattr(inst, "is_transpose", False):
                        last_w = None
                elif nm == "InstNoOp":
                    pass
                else:
                    last_w = None
                out.append(inst)
            if changed:
                bb.instructions = out
    return n_elided


NCORES = 8
N = 16384
SH = N // NCORES  # 2048 adj rows per core
F = 512
HID = 1024
OUT = 256

BF16 = mybir.dt.bfloat16
F32 = mybir.dt.float32
FP8 = mybir.dt.float8e4
ADJ_SCALE = float(N)  # adj pre-scaled into fp8 range; 1/N folded downstream
S2_SCALE = 1024.0  # s2 pre-scaled into fp8 range; undone at phase D evac
S_Z = 0.5  # zT fp8 scale (centered z, max ~100 < 240)
S_W1 = 64.0  # W1 fp8 scale
S_H = 8192.0  # centered-h fp8 scale (max ~126 < 240)
S_W2 = 64.0  # W2 fp8 scale

DR = mybir.MatmulPerfMode.DoubleRow

_built = None


def build():
    """Build the per-core Bass program (identical on all cores)."""
    nc = bass.Bass()

    # All big inputs are host-prepped into partition-major tiled layouts so
    # every DMA reads long contiguous per-partition runs (8-64 KiB):
    #   adjD[p, (k4 kk m)] = adjT[k4*512+kk*128+p, m]  (phases A and D)
    #   xP  [p, (kb f)]    = x[kb*128+p, f]            (replicated)
    adjA = nc.declare_dram_parameter("adjA", [128, 2 * 32 * 4 * 1024], FP8, isOutput=False)
    adjD = nc.declare_dram_parameter("adjD", [128, 32 * 4 * SH], FP8, isOutput=False)
    xP = nc.declare_dram_parameter("xP", [128, (N // 128) * F], FP8, isOutput=False)
    w1 = nc.declare_dram_parameter("w1", [F, HID], FP8, isOutput=False)
    w2 = nc.declare_dram_parameter("w2", [HID, OUT], FP8, isOutput=False)
    b1T = nc.declare_dram_parameter("b1T", [128, HID // 128], F32, isOutput=False)
    b2T = nc.declare_dram_parameter("b2T", [128, OUT // 128], F32, isOutput=False)
    # per-element subtraction at the zT evac: rank-1 x-quant correction
    # m_f*R_m PLUS the (broadcast) column mean of z, so the fp8 zT holds only
    # the zero-mean residual.  The exact mean returns via the phase-H bias.
    # bf16 (rounding noise ~0.04% of pre-act sigma): at 4 MB the fp32 version
    # monopolized HBM in the front-loaded first chunk and starved the adjA
    # stream.
    zcorr = nc.declare_dram_parameter("zcorr", [128, (F // 128) * SH], BF16, isOutput=False)
    # per-partition S_H*h~ (colmean-of-h estimate), subtracted after relu
    hcorrT = nc.declare_dram_parameter("hcorrT", [128, HID // 128], F32, isOutput=False)
    # S2_SCALE * (h~ @ W2), replicated across partitions, added at phase-S evac
    scorrT = nc.declare_dram_parameter("scorrT", [128, OUT], F32, isOutput=False)
    # [n2t, mb, p, m]: per-psum-bank contiguous so the final stores hit DMA
    # line rate (the [OUT, SH] layout made every partition row a strided
    # 2 KB piece and the stores trickled for ~5us past the last matmul)
    out2T = nc.declare_dram_parameter("out2T", [2, 4, 128, 512], F32, isOutput=True)

    rg = [list(range(NCORES))]

    def allgather(inp, outp):
        return nc.gpsimd.collective_compute(
            "AllGather",
            mybir.AluOpType.bypass,
            replica_groups=rg,
            ins=[inp.opt()],
            outs=[outp.opt()],
        )

    with tile.TileContext(nc) as tc:
        with (
            tc.tile_pool(name="const", bufs=1) as constp,
            tc.tile_pool(name="psum", bufs=8, space="PSUM") as psum,
            tc.tile_pool(name="dram", bufs=1, space="DRAM") as dram,
            tc.tile_pool(name="s2p", bufs=6) as s2p,
            tc.tile_pool(name="adjD", bufs=4) as adjDp,
        ):
            # ---- constants (ACT HWDGE ring; adj streams ride the SP ring).
            # Declared here, but the DMAs are issued AFTER the first x tiles
            # below: phase A's first matmul gates on x tile 0, while the
            # weights aren't read until phase H much later.
            w1t = constp.tile([128, F // 128, HID], FP8)
            w2t = constp.tile([128, HID // 128, OUT], FP8)
            b1t = constp.tile([128, HID // 128], F32)
            b2t = constp.tile([128, OUT // 128], F32)
            hct = constp.tile([128, HID // 128], F32)
            sct = constp.tile([128, OUT], F32)
            zct = constp.tile([128, 4, F // 128, 512], BF16)  # [p, chunk, fb, m]
            wz = constp.tile([128, 2, 512], FP8)  # zeroed warm-up operand

            # AG buffers partition-major: rank contribution [128, skk*256+n]
            # with s2 row skk*128+p; gathered output stacks ranks on dim 0.
            # Collectives cost 30-60us nearly size-independent AND serialize
            # on the single CC stream, so one quarter-gather fires per
            # m-chunk (~41us apart): even worst-case collectives drain
            # before phase D's quarter-major consumption needs them.
            ag_in = [dram.tile([128, 4 * OUT], FP8, name=f"agi{q}") for q in range(4)]
            ag_out = [
                dram.tile([128 * 8, 4 * OUT], FP8, addr_space="Shared", name=f"ago{q}")
                for q in range(4)
            ]

            # xP holds SW-interleaved stationary blocks for phase A:
            #   [p, q, fb, (127-c)*2 + ko] = x_hi[q*256 + ko*128 + p, fb*128 + c]
            xsrc = xP[:].rearrange("p (q fb c) -> p q fb c", q=N // 256, fb=F // 128)

            # ag_out readers: [p, g, skk, n]; s2 global row for quarter qq
            # is g*2048 + qq*512 + skk*128 + p
            s2srcs = [
                ag_out[qq][:].rearrange("(g p) (skk n) -> p g skk n", p=128, n=OUT)
                for qq in range(4)
            ]
            dsrc = adjD[:].rearrange("p (k4 kk m) -> p k4 kk m", k4=32, kk=4)
            # phase D k-step order: quarter-major, matching AG firing order
            k4_order = [g * 4 + qq for qq in range(4) for g in range(8)]
            st_pre = {}
            ad_pre = {}

            with (
                tc.tile_pool(name="xp", bufs=1) as xp,
                tc.tile_pool(name="zt", bufs=16) as ztp,
                tc.tile_pool(name="ht", bufs=16) as htp,
                tc.tile_pool(name="htmp", bufs=4) as htmpp,
                tc.tile_pool(name="adjA", bufs=6) as adjp,
                tc.tile_pool(name="small", bufs=4) as smallp,
            ):
                # ---- PE warm-up: the engine preamble ends ~10.5us and the
                # first adjA/x bytes land ~14us; 7 throwaway fp8-DR matmuls
                # on a zeroed tile fill that window and lift the HAM clock
                # gate (needs ~3.4us of sustained PE busy) so the leading
                # real matmuls run at 2.4 GHz instead of 1.2.  Results land
                # in psum banks phase A's start=True matmuls overwrite.
                # memset rides the otherwise-idle GpSimd queue.
                with tc.high_priority():
                    nc.gpsimd.memset(wz[:], 0)
                    wps = [
                        psum.tile([128, 512], F32, tag="ps", name=f"psW{i}")
                        for i in range(2)
                    ]
                    for i in range(7):
                        nc.tensor.matmul(
                            wps[i % 2][:],
                            wz[:, :, 0:128],
                            wz[:],
                            start=True,
                            stop=True,
                            perf_mode=DR,
                        )

                xts = []  # 16 tiles of 8 k-blocks each
                zt = {}
                ht = {}
                aAv = adjA[:].rearrange(
                    "p (c k4 kk m) -> p c k4 kk m", c=4, k4=32, kk=4
                )
                act_scale = S_H / (S_Z * S_W1 * ADJ_SCALE)
                s_evac = S2_SCALE / (S_H * S_W2)
                for ch in range(4):
                    asrc = aAv[:, ch]
                    # ---- phase A: zT chunk = (adj_c @ x)^T cols ch*512.. --
                    ps = [
                        psum.tile([128, 512], F32, tag="ps", name=f"psA{ch}{i}")
                        for i in range(4)
                    ]
                    for k4 in range(32):
                        if ch == 0 and k4 % 2 == 0:
                            i = k4 // 2
                            t = xp.tile([128, 4, 4, 256], FP8, name=f"xt{i}")
                            if i == 0:
                                # fine-grained first transfer: the leading
                                # matmuls need only q=0
                                for qi in range(4):
                                    nc.scalar.dma_start(t[:, qi], xsrc[:, qi])
                            else:
                                nc.scalar.dma_start(
                                    t[:], xsrc[:, i * 4 : (i + 1) * 4]
                                )
                            xts.append(t)
                            if i >= 9 and i % 2 == 1:
                                # one 512 KB zcorr chunk-piece rides between
                                # late x tiles (the stream has banked slack
                                # by then; earlier slots delayed tight first
                                # loads).  As one trailing 2-4 MB transfer it
                                # finished ~9us after the first chunk's evac
                                # needed it, stalling the whole pipeline.
                                zch = (i - 9) // 2
                                nc.scalar.dma_start(
                                    zct[:, zch],
                                    zcorr[:].rearrange(
                                        "p (c fb mm) -> p c fb mm", c=4, fb=F // 128
                                    )[:, zch],
                                )
                            if i == 15:
                                # x fully queued; now the weight constants
                                nc.scalar.dma_start(
                                    w1t[:],
                                    w1[:].rearrange("(fb p) j -> p fb j", p=128),
                                )
                                nc.scalar.dma_start(
                                    w2t[:],
                                    w2[:].rearrange("(jb p) n -> p jb n", p=128),
                                )
                                nc.scalar.dma_start(b1t[:], b1T[:])
                                nc.scalar.dma_start(b2t[:], b2T[:])
                                nc.scalar.dma_start(hct[:], hcorrT[:])
                                nc.scalar.dma_start(sct[:], scorrT[:])
                        if ch == 3 and k4 == 16:
                            # prefetch phase D's first four s2 tiles (g 0..3
                            # of quarter 0).  That AllGather completed long
                            # ago, so the SP queue won't block, and phase
                            # D's first matmuls start without a JIT-load
                            # bubble.
                            for pk in range(4):
                                stp = s2p.tile(
                                    [128, 4, OUT], FP8, tag="s2t", bufs=6,
                                    name=f"s2tp{pk}",
                                )
                                nc.sync.dma_start(stp[:], s2srcs[0][:, pk])
                                st_pre[pk] = stp
                        if ch == 3 and k4 in (20, 26):
                            # prefetch phase D's first adjD tiles: the ACT
                            # engine reaches these dma_starts mid-A (its
                            # queue is idle then), so the 1 MB loads land
                            # well before phase D starts instead of being
                            # issued behind the last H evacs.
                            pk = 0 if k4 == 20 else 1
                            adp = adjDp.tile(
                                [128, 4, SH], FP8, tag="adjD", bufs=4,
                                name=f"aDp{pk}",
                            )
                            nc.scalar.dma_start(adp[:], dsrc[:, k4_order[pk]])
                            ad_pre[pk] = adp
                        at = adjp.tile(
                            [128, 4, 512], FP8, tag="adjA", bufs=6, name=f"aA{ch}{k4}"
                        )
                        if ch == 0 and k4 < 2:
                            # fine-grained: j2=0 matmuls need only kk 0:2
                            nc.sync.dma_start(at[:, 0:2], asrc[:, k4, 0:2])
                            nc.sync.dma_start(at[:, 2:4], asrc[:, k4, 2:4])
                        else:
                            nc.sync.dma_start(at[:], asrc[:, k4])
                        # fp8 DoubleRow: contraction 256 rows per matmul
                        # (ki = partition, ko = kk-pair), 2x FLOP rate.
                        for j2 in range(2):
                            q = k4 * 2 + j2
                            xt = xts[q // 4]
                            for fb in range(4):
                                nc.tensor.matmul(
                                    ps[fb][:],
                                    xt[:, q % 4, fb, :],
                                    at[:, 2 * j2 : 2 * j2 + 2, :],
                                    start=(q == 0),
                                    stop=(q == 63),
                                    perf_mode=mybir.MatmulPerfMode.DoubleRowSwInterleave,
                                )
                    for fb in range(4):
                        if fb % 2 == 0:
                            zz = ztp.tile(
                                [128, 2, 512], FP8, tag="zt", bufs=16,
                                name=f"zt{ch}{fb}",
                            )
                            zt[(fb // 2, ch)] = zz
                        # fp8 zT holds S_Z*(psum - zcorr): zero-mean
                        # residual of z, coherent means handled exactly
                        nc.vector.scalar_tensor_tensor(
                            zt[(fb // 2, ch)][:, fb % 2, :],
                            ps[fb][:],
                            S_Z,
                            zct[:, ch, fb, :],
                            mybir.AluOpType.mult,
                            mybir.AluOpType.subtract,
                        )

                    # ---- phase H: hT chunk = relu(z @ W1 + b1)^T, fp8 DR --
                    # psum_H = (S_Z*deltaN) @ (S_W1*W1) = S_Z*S_W1*N*(delta@W1)
                    for jbh in range(2):
                        psh = [
                            psum.tile([128, 512], F32, tag="ps", name=f"psH{ch}{jbh}{i}")
                            for i in range(4)
                        ]
                        for jb in range(4):
                            jg = jbh * 4 + jb
                            for fbp in range(2):
                                nc.tensor.matmul(
                                    psh[jb][:],
                                    w1t[:, 2 * fbp : 2 * fbp + 2, jg * 128 : (jg + 1) * 128],
                                    zt[(fbp, ch)][:],
                                    start=(fbp == 0),
                                    stop=(fbp == 1),
                                    perf_mode=DR,
                                )
                            # tmp = S_H*relu(...): ACT does scale+bias+relu;
                            # DVE then subtracts S_H*h~ and casts to fp8
                            tmp = htmpp.tile(
                                [128, 512], BF16, tag="htmp", bufs=4,
                                name=f"htm{ch}{jbh}{jb}",
                            )
                            nc.scalar.activation(
                                tmp[:],
                                psh[jb][:],
                                mybir.ActivationFunctionType.Relu,
                                bias=b1t[:, jg : jg + 1],
                                scale=act_scale,
                            )
                            if jg % 2 == 0:
                                hh = htp.tile(
                                    [128, 2, 512], FP8, tag="ht", bufs=16,
                                    name=f"ht{ch}{jbh}{jb}",
                                )
                                ht[(jg // 2, ch)] = hh
                            nc.vector.tensor_scalar(
                                ht[(jg // 2, ch)][:, jg % 2, :],
                                tmp[:],
                                1.0,
                                hct[:, jg : jg + 1],
                                mybir.AluOpType.mult,
                                mybir.AluOpType.subtract,
                            )

                    # ---- phase S: s2 chunk = h @ W2, fp8 DR; AG/quarter ---
                    # psum_S = S_H*S_W2*((h-h~)@W2); evac adds S2_SCALE*h~@W2
                    for m4 in range(4):
                        pss = psum.tile([128, 256], F32, tag="ps", name=f"psS{ch}{m4}")
                        for jp in range(4):
                            nc.tensor.matmul(
                                pss[:],
                                ht[(jp, ch)][:, :, m4 * 128 : (m4 + 1) * 128],
                                w2t[:, 2 * jp : 2 * jp + 2, :],
                                start=(jp == 0),
                                stop=(jp == 3),
                                perf_mode=DR,
                            )
                        so = smallp.tile([128, 256], FP8, tag="so", bufs=4)
                        nc.vector.scalar_tensor_tensor(
                            so[:],
                            pss[:],
                            s_evac,
                            sct[:],
                            mybir.AluOpType.mult,
                            mybir.AluOpType.add,
                        )
                        # ACT HWDGE ring: the gpsimd queue must stay empty
                        # of data movement — a collective trigger BLOCKS it
                        # until the CC stream is free, and anything queued
                        # behind cascades into a PE stall.
                        nc.scalar.dma_start(
                            ag_in[ch][:, m4 * OUT : (m4 + 1) * OUT], so[:]
                        )
                    allgather(ag_in[ch], ag_out[ch])

            # ---- phase D: out2T = (adj_c @ s2)^T + b2 ----
            # All 8 psum banks accumulate concurrently; k-blocks consumed in
            # gather-arrival order (quarter-major), s2 tiles loaded JIT after
            # each adjT chunk so the SP queue stays load-ordered.
            with (
                tc.tile_pool(name="outp", bufs=8) as outp,
            ):
                dps = [
                    psum.tile([128, 512], F32, tag="ps", name=f"psD{i}")
                    for i in range(8)
                ]
                inv = 1.0 / (ADJ_SCALE * S2_SCALE)

                def d_evac(n2t, mb):
                    # scalar and vector engines each take half (they can
                    # access PSUM concurrently on different banks), stores
                    # split across both HWDGE rings
                    ot = outp.tile([128, 512], F32, tag="ot")
                    if mb % 2 == 0:
                        nc.scalar.activation(
                            ot[:],
                            dps[n2t * 4 + mb][:],
                            mybir.ActivationFunctionType.Identity,
                            bias=b2t[:, n2t : n2t + 1],
                            scale=inv,
                        )
                    else:
                        nc.vector.tensor_scalar(
                            ot[:],
                            dps[n2t * 4 + mb][:],
                            inv,
                            b2t[:, n2t : n2t + 1],
                            mybir.AluOpType.mult,
                            mybir.AluOpType.add,
                        )
                    dmaq = nc.scalar if mb % 2 == 0 else nc.sync
                    dmaq.dma_start(out2T[n2t, mb], ot[:])

                for ki, k4 in enumerate(k4_order):
                    g, qq = k4 // 4, k4 % 4
                    if ki in ad_pre:
                        at = ad_pre[ki]
                    else:
                        at = adjDp.tile(
                            [128, 4, SH], FP8, tag="adjD", bufs=4, name=f"aD{k4}"
                        )
                        nc.scalar.dma_start(at[:], dsrc[:, k4])
                    if ki in st_pre:
                        st = st_pre[ki]
                    else:
                        st = s2p.tile(
                            [128, 4, OUT], FP8, tag="s2t", bufs=6, name=f"s2t{k4}"
                        )
                        nc.sync.dma_start(st[:], s2srcs[qq][:, g])
                    if ki < 31:
                        for j2 in range(2):
                            for n2t in range(2):
                                lhs = st[:, 2 * j2 : 2 * j2 + 2, n2t * 128 : (n2t + 1) * 128]
                                for mb in range(4):
                                    nc.tensor.matmul(
                                        dps[n2t * 4 + mb][:],
                                        lhs,
                                        at[:, 2 * j2 : 2 * j2 + 2, mb * 512 : (mb + 1) * 512],
                                        start=(ki == 0 and j2 == 0),
                                        stop=False,
                                        perf_mode=DR,
                                    )
                    else:
                        # final K-step bank-major: each bank finishes its
                        # accumulation then evacs+stores immediately, hiding
                        # the output tail under the remaining matmuls
                        for n2t in range(2):
                            for mb in range(4):
                                for j2 in range(2):
                                    lhs = st[:, 2 * j2 : 2 * j2 + 2, n2t * 128 : (n2t + 1) * 128]
                                    nc.tensor.matmul(
                                        dps[n2t * 4 + mb][:],
                                        lhs,
                                        at[:, 2 * j2 : 2 * j2 + 2, mb * 512 : (mb + 1) * 512],
                                        start=False,
                                        stop=(j2 == 1),
                                        perf_mode=DR,
                                    )
                                d_evac(n2t, mb)

    _elide_redundant_ldweights(nc)
    _split_excess_waits(nc)
    return nc


def _erf(z):
    """Abramowitz-Stegun 7.1.26 erf approximation (|err| < 1.5e-7)."""
    s = np.sign(z)
    z = np.abs(z)
    t = 1.0 / (1.0 + 0.3275911 * z)
    poly = t * (
        0.254829592
        + t * (-0.284496736 + t * (1.421413741 + t * (-1.453152027 + t * 1.061405429)))
    )
    return s * (1.0 - poly * np.exp(-z * z))


def _prep_inputs(x, adj, W1, b1, W2, b2):
    e4 = ml_dtypes.float8_e4m3

    def q8(a):
        return np.clip(a, -240.0, 240.0).astype(e4)

    # Phase A runs in fp8 (DoubleRow): adj pre-scaled by N into e4m3 range,
    # 1/N folded downstream.  x quantized to e4m3; the coherent part of its
    # quantization error (rank-1: adjq_rowsum x colmean(x_hi - x)) is
    # cancelled in the zT evac together with the z column mean.
    x_hi = q8(x)
    x_hi32 = x_hi.astype(np.float32)
    # SW-interleaved stationary layout for DoubleRowSwInterleave:
    #   xP[p, q, fb, (127-c)*2+ko] = x_hi[q*256 + ko*128 + p, fb*128 + c]
    a5x = x_hi.reshape(N // 256, 2, 128, F // 128, 128)  # [q, ko, ki, fb, c]
    xb = np.ascontiguousarray(
        a5x.transpose(2, 0, 3, 4, 1)[:, :, :, ::-1, :]
    ).reshape(128, -1)
    m = (x_hi32 - x).mean(axis=0)  # [F]
    b2T = np.ascontiguousarray(b2.reshape(OUT // 128, 128).T).astype(np.float32)

    adjTqs = []
    colsum = np.zeros(N, dtype=np.float64)  # sum over adj ROWS of adjq [k]
    rrsum = 0.0
    for c in range(NCORES):
        rows = slice(c * SH, (c + 1) * SH)
        # adjT[k, m] = adj[c*SH + m, k], shape [N, SH], k-major
        adjTq = q8(np.ascontiguousarray(adj[rows, :].T) * ADJ_SCALE)
        adjTqs.append(adjTq)
        aq32 = adjTq.astype(np.float32)
        colsum += aq32.sum(axis=1, dtype=np.float64)
        rrsum += float(aq32.sum(dtype=np.float64))
    # muN[f] = colmean over all rows of (adjq@xq - rr*m)  (N-scaled z)
    muN = (colsum / N).astype(np.float32) @ x_hi32 - m * np.float32(rrsum / N)

    # phase-H effective bias (true scale) and its Gaussian h-mean estimate
    bias_eff = (muN / ADJ_SCALE) @ W1 + b1  # [HID]
    var_delta_f = (1.0 / 12.0) * (x * x).sum(axis=0) / (ADJ_SCALE * ADJ_SCALE)
    sig_j = np.sqrt(np.maximum(var_delta_f @ (W1 * W1), 1e-30))  # [HID]
    t = bias_eff / sig_j
    phi_cdf = 0.5 * (1.0 + _erf(t / np.sqrt(2.0)))
    phi_pdf = np.exp(-0.5 * t * t) / np.sqrt(2.0 * np.pi)
    h_tilde = bias_eff * phi_cdf + sig_j * phi_pdf  # ~ colmean(relu(pre))

    b1T_eff = np.ascontiguousarray(
        (S_H * bias_eff).reshape(HID // 128, 128).T
    ).astype(np.float32)
    hcorrT = np.ascontiguousarray(
        (S_H * h_tilde).reshape(HID // 128, 128).T
    ).astype(np.float32)
    scorr = (S2_SCALE * (h_tilde @ W2)).astype(np.float32)  # [OUT]
    scorrT = np.broadcast_to(scorr, (128, OUT)).copy()

    w1q = q8(S_W1 * W1)
    w2q = q8(S_W2 * W2)

    in_maps = []
    for c in range(NCORES):
        adjTq = adjTqs[c]
        a5 = adjTq.reshape(32, 4, 128, 4, 512)  # [k4, kk, p, chunk, m]
        adjA_ = np.ascontiguousarray(a5.transpose(2, 3, 0, 1, 4)).reshape(128, -1)
        d4 = adjTq.reshape(32, 4, 128, SH)  # [k4, kk, p, m]
        adjD_ = np.ascontiguousarray(d4.transpose(2, 0, 1, 3)).reshape(128, -1)
        rr = adjTq.astype(np.float32).sum(axis=0)  # [SH] adjq rowsums
        # zcorr = S_Z*(m_f*rr_m + muN_f): x-quant rank-1 + z column mean
        # layout [p, chunk, fb, m512] so each chunk's piece is contiguous
        zc = (S_Z * (m[:, None] * rr[None, :] + muN[:, None])).reshape(
            F // 128, 128, 4, 512
        )
        zcorr_ = (
            np.ascontiguousarray(zc.transpose(1, 2, 0, 3)).reshape(128, -1)
        ).astype(ml_dtypes.bfloat16)
        in_maps.append(
            {
                "adjA": adjA_,
                "adjD": adjD_,
                "xP": xb,
                "w1": w1q,
                "w2": w2q,
                "b1T": b1T_eff,
                "b2T": b2T,
                "zcorr": zcorr_,
                "hcorrT": hcorrT,
                "scorrT": scorrT,
            }
        )
    return in_maps


def _run(inputs, trace=False):
    global _built
    if _built is None:
        _built = build()
    in_maps = _prep_inputs(**inputs)
    r = run_bass_kernel_spmd(_built, in_maps, list(range(NCORES)), trace=trace)
    out = np.empty([N, OUT], np.float32)
    for c in range(NCORES):
        # out2T[n2t, mb, p, m] = out[c*SH + mb*512 + m, n2t*128 + p]
        o4 = r.results[c]["out2T"]
        out[c * SH : (c + 1) * SH, :] = o4.transpose(1, 3, 0, 2).reshape(SH, OUT)
    return out, r


def kernel(x, adj, W1, b1, W2, b2):
    out, _ = _run(dict(x=x, adj=adj, W1=W1, b1=b1, W2=W2, b2=b2))
    return out
